# revision 16
# baseline (speedup 1.0000x reference)
"""GQA attention decode kernel (B=16,S=16,D=4096,H=32,KV=8,HD=128,T=4096) on 8 TRN2 cores.

Tensor-parallel sharding: core c owns kv-head c and q-heads 4c..4c+3.
x/k/v replicated; wq/wk/wv/wo and KV caches sharded by head; chunked
ReduceScatter over the output projection; host stitches per-core shards.

Note: each PSUM matmul-accumulation chain must own its tile — interleaved
chains into column sub-ranges of one PSUM bank produce wrong results.
"""

import os
import numpy as np

import concourse.bass as bass
import concourse.bacc as bacc
import concourse.tile as tile
import concourse.mybir as mybir
from concourse import masks
from concourse.bass_utils import run_bass_kernel_spmd

dt = mybir.dt
F32 = dt.float32
BF16 = dt.bfloat16

B, S, D = 16, 16, 4096
H, KV, HD = 32, 8, 128
MAX_S = 4096
START = 4080
T = START + S           # 4096
N_CORES = 8
TOK = B * S             # 256 tokens
HPC = H // N_CORES      # 4 q-heads per core
QD = HPC * HD           # 512 per-core q dims
NT = T // 128           # 32 t-tiles
ND = D // 128           # 32 d-tiles
SCALE = 1.0 / float(np.sqrt(HD))

_CACHE = {}
DEBUG = bool(int(os.environ.get("KERNEL_DEBUG", "0")))


def _build():
    nc = bacc.Bacc("TRN2", target_bir_lowering=False, debug=False,
                   num_devices=N_CORES)

    xe = nc.declare_dram_parameter("x", [TOK, D], F32, isOutput=False)
    ke = nc.declare_dram_parameter("k", [TOK, D], F32, isOutput=False)
    ve = nc.declare_dram_parameter("v", [TOK, D], F32, isOutput=False)
    wqe = nc.declare_dram_parameter("wq", [D, QD], F32, isOutput=False)
    wke = nc.declare_dram_parameter("wk", [D, HD], F32, isOutput=False)
    wve = nc.declare_dram_parameter("wv", [D, HD], F32, isOutput=False)
    woe = nc.declare_dram_parameter("wo", [QD, D], F32, isOutput=False)
    cke = nc.declare_dram_parameter("ck", [B, T, HD], F32, isOutput=False)
    cve = nc.declare_dram_parameter("cv", [B, T, HD], F32, isOutput=False)
    cose = nc.declare_dram_parameter("cos", [128, HD // 2], F32, isOutput=False)
    sine = nc.declare_dram_parameter("sin", [128, HD // 2], F32, isOutput=False)
    oute = nc.declare_dram_parameter("out", [2 * S, D], F32, isOutput=True)
    if DEBUG:
        dbg_q = nc.declare_dram_parameter("dbg_q", [128, HPC * TOK], F32,
                                          isOutput=True)
        dbg_kT = nc.declare_dram_parameter("dbg_kT", [128, T], F32,
                                           isOutput=True)
        dbg_pr = nc.declare_dram_parameter("dbg_pr", [128, NT * 64], F32,
                                           isOutput=True)
        dbg_at = nc.declare_dram_parameter("dbg_at", [128, 8 * 128], F32,
                                           isOutput=True)
        dbg_kn = nc.declare_dram_parameter("dbg_kn", [128, TOK], F32,
                                           isOutput=True)
        dbg_xv = nc.declare_dram_parameter("dbg_xv", [128, 2 * HD], F32,
                                           isOutput=True)

    with tile.TileContext(nc) as tc:
        with (
            tc.tile_pool(name="const", bufs=1) as const,
            tc.tile_pool(name="stage", bufs=4) as stage,      # [128,4096] f32
            tc.tile_pool(name="pstage", bufs=2) as pstage,    # phase1 x/k/v chunks
            tc.tile_pool(name="wf", bufs=4) as wf,
            tc.tile_pool(name="wb", bufs=4) as wb,
            tc.tile_pool(name="xtp", bufs=4) as xtp,
            tc.tile_pool(name="ktp", bufs=2) as ktp,          # K^T bf16
            tc.tile_pool(name="vbp", bufs=2) as vbp,          # V bf16
            tc.tile_pool(name="prp", bufs=2) as prp,          # probs bf16
            tc.tile_pool(name="dnp", bufs=2) as dnp,          # denom scratch
            tc.tile_pool(name="ysb", bufs=3) as ysb,          # y staging
            tc.tile_pool(name="dram", bufs=1, space="DRAM") as dram,
        ):
            # ---- constants ----
            id32 = const.tile([128, 128], F32, name="id32")
            masks.make_identity(nc, id32[:])
            id16 = const.tile([128, 128], BF16, name="id16")
            masks.make_identity(nc, id16[:])
            ones16 = const.tile([128, 128], BF16, name="ones16")
            nc.gpsimd.memset(ones16[:], 1.0)
            cos_sb = const.tile([128, HD // 2], F32, name="cos_sb")
            sin_sb = const.tile([128, HD // 2], F32, name="sin_sb")
            nc.sync.dma_start(cos_sb[:], cose[:, :])
            nc.sync.dma_start(sin_sb[:], sine[:, :])

            # persistent activations
            xq_rope = const.tile([128, 2 * QD], BF16, name="xq_rope")
            xk_rope = const.tile([128, 2 * HD], BF16, name="xk_rope")
            xv_bf = const.tile([128, 2 * HD], BF16, name="xv_bf")
            q_T = const.tile([128, HPC * TOK], BF16, name="q_T")     # [hd,(h,tok)]
            kn_T = const.tile([128, TOK], BF16, name="kn_T")         # [hd,tok]
            wo_bf = const.tile([128, HPC * D], BF16, name="wo_bf")   # [hd,(h,d)]
            attnT = [
                const.tile([128, 128], BF16, name=f"attnT{ch}_{h}")
                for ch in range(2) for h in range(HPC)
            ]  # attnT[ch*HPC+h]: [hd, tok(128)] for chunk ch, head h

            # DRAM bounce buffers for the output-projection ReduceScatter
            yb = [dram.tile([128, D], F32, name=f"yb{ch}", tag=f"yb{ch}")
                  for ch in range(2)]
            rs = [dram.tile([S, D], F32, name=f"rs{ch}", tag=f"rs{ch}")
                  for ch in range(2)]

            # ================= phase 1: QKV projection =================
            with (
                tc.tile_pool(name="ps1T", bufs=2, space="PSUM") as ps1T,
                tc.tile_pool(name="ps1Q", bufs=2, space="PSUM") as ps1Q,
                tc.tile_pool(name="ps1K", bufs=2, space="PSUM") as ps1K,
                tc.tile_pool(name="ps1V", bufs=1, space="PSUM") as ps1V,
            ):
                # one PSUM tile per accumulation chain
                xq_ps0 = ps1Q.tile([128, 512], F32, name="xq_ps0", tag="q")
                xq_ps1 = ps1Q.tile([128, 512], F32, name="xq_ps1", tag="q")
                xk_ps0 = ps1K.tile([128, 128], F32, name="xk_ps0", tag="k")
                xk_ps1 = ps1K.tile([128, 128], F32, name="xk_ps1", tag="k")
                xvT_ps = ps1V.tile([128, 256], F32, name="xvT_ps", tag="v")

                x3 = [xe, ke, ve]
                for dd in range(ND):
                    # load [256,128] column slices of x/k/v as [128,(tt,128)]
                    srcs = []
                    for si, ext in enumerate(x3):
                        st = pstage.tile([128, 256], F32, name=f"p{si}_{dd}",
                                         tag=f"p{si}")
                        nc.sync.dma_start(
                            st[:].rearrange("p (a c) -> p a c", a=2),
                            ext[:, dd * 128:(dd + 1) * 128]
                            .rearrange("(a p) c -> p a c", p=128),
                        )
                        srcs.append(st)
                    # transpose x/k blocks (v stays un-transposed: xv uses
                    # the transposed-output projection, rhs = v^T... rhs must
                    # be vT: transpose v too)
                    tA = ps1T.tile([128, 512], F32, name=f"tA_{dd}", tag="t")
                    tB = ps1T.tile([128, 512], F32, name=f"tB_{dd}", tag="t")
                    for tt in range(2):
                        nc.tensor.transpose(tA[:, tt * 128:(tt + 1) * 128],
                                            srcs[0][:, tt * 128:(tt + 1) * 128],
                                            id32[:])
                        nc.tensor.transpose(tA[:, 256 + tt * 128:256 + (tt + 1) * 128],
                                            srcs[1][:, tt * 128:(tt + 1) * 128],
                                            id32[:])
                        nc.tensor.transpose(tB[:, tt * 128:(tt + 1) * 128],
                                            srcs[2][:, tt * 128:(tt + 1) * 128],
                                            id32[:])
                    xt_dd = xtp.tile([128, 768], BF16, name=f"xt_{dd}", tag="xt")
                    nc.vector.tensor_copy(xt_dd[:, 0:512], tA[:, 0:512])
                    nc.vector.tensor_copy(xt_dd[:, 512:768], tB[:, 0:256])

                    # weights for this d-tile
                    wq_f = wf.tile([128, QD], F32, name=f"wqf_{dd}", tag="wqf")
                    nc.sync.dma_start(wq_f[:], wqe[dd * 128:(dd + 1) * 128, :])
                    wq_b = wb.tile([128, QD], BF16, name=f"wqb_{dd}", tag="wqb")
                    nc.scalar.activation(wq_b[:], wq_f[:],
                                         mybir.ActivationFunctionType.Copy)
                    wk_f = wf.tile([128, HD], F32, name=f"wkf_{dd}", tag="wkf")
                    nc.sync.dma_start(wk_f[:], wke[dd * 128:(dd + 1) * 128, :])
                    wk_b = wb.tile([128, HD], BF16, name=f"wkb_{dd}", tag="wkb")
                    nc.scalar.activation(wk_b[:], wk_f[:],
                                         mybir.ActivationFunctionType.Copy)
                    wv_f = wf.tile([128, HD], F32, name=f"wvf_{dd}", tag="wvf")
                    nc.sync.dma_start(wv_f[:], wve[dd * 128:(dd + 1) * 128, :])
                    wv_b = wb.tile([128, HD], BF16, name=f"wvb_{dd}", tag="wvb")
                    nc.scalar.activation(wv_b[:], wv_f[:],
                                         mybir.ActivationFunctionType.Copy)

                    fl = dict(start=(dd == 0), stop=(dd == ND - 1))
                    nc.tensor.matmul(xq_ps0[:], xt_dd[:, 0:128], wq_b[:], **fl)
                    nc.tensor.matmul(xq_ps1[:], xt_dd[:, 128:256], wq_b[:], **fl)
                    nc.tensor.matmul(xk_ps0[:], xt_dd[:, 256:384], wk_b[:], **fl)
                    nc.tensor.matmul(xk_ps1[:], xt_dd[:, 384:512], wk_b[:], **fl)
                    # xv^T = wv^T @ v  (single chain, N=256)
                    nc.tensor.matmul(xvT_ps[:], wv_b[:], xt_dd[:, 512:768], **fl)

                # ---- RoPE on xq / xk ----
                rp = dnp  # small scratch pool
                for tt in range(2):
                    xq_ps = (xq_ps0, xq_ps1)[tt]
                    for h in range(HPC):
                        src = xq_ps[:].rearrange("p (h i two) -> p h i two",
                                                 h=HPC, two=2)
                        x0 = src[:, h, :, 0]
                        x1 = src[:, h, :, 1]
                        dst = xq_rope[:, tt * QD:(tt + 1) * QD].rearrange(
                            "p (h i two) -> p h i two", h=HPC, two=2)
                        r0 = dst[:, h, :, 0]
                        r1 = dst[:, h, :, 1]
                        t0 = rp.tile([128, 64], F32, name=f"t0_{tt}_{h}", tag="t0")
                        t1 = rp.tile([128, 64], F32, name=f"t1_{tt}_{h}", tag="t1")
                        nc.vector.tensor_mul(t0[:], x0, cos_sb[:])
                        nc.vector.tensor_mul(t1[:], x1, sin_sb[:])
                        nc.vector.tensor_sub(r0, t0[:], t1[:])
                        nc.vector.tensor_mul(t0[:], x0, sin_sb[:])
                        nc.vector.tensor_mul(t1[:], x1, cos_sb[:])
                        nc.vector.tensor_add(r1, t0[:], t1[:])
                    # xk rope
                    xk_ps = (xk_ps0, xk_ps1)[tt]
                    srck = xk_ps[:].rearrange("p (i two) -> p i two", two=2)
                    k0 = srck[:, :, 0]
                    k1 = srck[:, :, 1]
                    dstk = xk_rope[:, tt * HD:(tt + 1) * HD].rearrange(
                        "p (i two) -> p i two", two=2)
                    kr0 = dstk[:, :, 0]
                    kr1 = dstk[:, :, 1]
                    t0 = rp.tile([128, 64], F32, name=f"kt0_{tt}", tag="t0")
                    t1 = rp.tile([128, 64], F32, name=f"kt1_{tt}", tag="t1")
                    nc.vector.tensor_mul(t0[:], k0, cos_sb[:])
                    nc.vector.tensor_mul(t1[:], k1, sin_sb[:])
                    nc.vector.tensor_sub(kr0, t0[:], t1[:])
                    nc.vector.tensor_mul(t0[:], k0, sin_sb[:])
                    nc.vector.tensor_mul(t1[:], k1, cos_sb[:])
                    nc.vector.tensor_add(kr1, t0[:], t1[:])

                # xv: cast ^T result to bf16 then PE-transpose back to natural
                xvT_bf = const.tile([128, 256], BF16, name="xvT_bf")
                nc.vector.tensor_copy(xvT_bf[:], xvT_ps[:])
                xvn = ps1T.tile([128, 512], BF16, name="xvn", tag="t")
                for tt in range(2):
                    nc.tensor.transpose(xvn[:, tt * 128:(tt + 1) * 128],
                                        xvT_bf[:, tt * 128:(tt + 1) * 128],
                                        id16[:])
                nc.vector.tensor_copy(xv_bf[:], xvn[:, 0:256])

                # ---- build q_T [hd,(h,tok)] and kn_T [hd,tok] ----
                for tt in range(2):
                    qtp = ps1T.tile([128, 512], BF16, name=f"qtp_{tt}", tag="t")
                    for h in range(HPC):
                        nc.tensor.transpose(
                            qtp[:, h * 128:(h + 1) * 128],
                            xq_rope[:, tt * QD + h * 128:tt * QD + (h + 1) * 128],
                            id16[:])
                    for h in range(HPC):
                        nc.vector.tensor_copy(
                            q_T[:, h * TOK + tt * 128:h * TOK + (tt + 1) * 128],
                            qtp[:, h * 128:(h + 1) * 128])
                ktp_ps = ps1T.tile([128, 512], BF16, name="ktp_ps", tag="t")
                for tt in range(2):
                    nc.tensor.transpose(ktp_ps[:, tt * 128:(tt + 1) * 128],
                                        xk_rope[:, tt * HD:(tt + 1) * HD],
                                        id16[:])
                nc.vector.tensor_copy(kn_T[:], ktp_ps[:, 0:256])

            qv = q_T[:].rearrange("p (h t) -> p h t", h=HPC)

            if DEBUG:
                dq = stage.tile([128, HPC * TOK], F32, name="dq", tag="st")
                nc.vector.tensor_copy(dq[:, 0:HPC * TOK], q_T[:])
                nc.sync.dma_start(dbg_q[:, :], dq[:, 0:HPC * TOK])
                dkn = dnp.tile([128, TOK], F32, name="dkn", tag="dkn")
                nc.vector.tensor_copy(dkn[:], kn_T[:])
                nc.sync.dma_start(dbg_kn[:, :], dkn[:])
                dxv = dnp.tile([128, 2 * HD], F32, name="dxv", tag="dxv")
                nc.vector.tensor_copy(dxv[:], xv_bf[:])
                nc.sync.dma_start(dbg_xv[:, :], dxv[:])

            # ================= phase 2: attention over batches =============
            with (
                tc.tile_pool(name="psA", bufs=2, space="PSUM") as psA,
                tc.tile_pool(name="psB", bufs=3, space="PSUM") as psB,
                tc.tile_pool(name="psC", bufs=3, space="PSUM") as psC,
            ):
                def do_batch(b):
                    ch = b // 8
                    col = (b % 8) * 16  # column offset inside chunk buffers

                    k_sb = stage.tile([128, T], F32, name=f"ksb_{b}", tag="st")
                    nc.sync.dma_start(
                        k_sb[:].rearrange("p (a c) -> p a c", a=NT),
                        cke[b].rearrange("(a p) c -> p a c", p=128))
                    v_sb = stage.tile([128, T], F32, name=f"vsb_{b}", tag="st")
                    nc.sync.dma_start(
                        v_sb[:].rearrange("p (a c) -> p a c", a=NT),
                        cve[b].rearrange("(a p) c -> p a c", p=128))

                    # K^T (bf16) via PE transpose + cast-copy
                    kT = ktp.tile([128, T], BF16, name=f"kT_{b}", tag="kT")
                    for g in range(8):
                        tp = psA.tile([128, 512], F32, name=f"tp_{b}_{g}",
                                      tag="a")
                        for j in range(4):
                            tt = g * 4 + j
                            nc.tensor.transpose(
                                tp[:, j * 128:(j + 1) * 128],
                                k_sb[:, tt * 128:(tt + 1) * 128], id32[:])
                        if g < 7:
                            nc.vector.tensor_copy(
                                kT[:, g * 512:(g + 1) * 512], tp[:])
                        else:
                            nc.vector.tensor_copy(
                                kT[:, 3584:4080], tp[:, 0:496])
                    # patch new keys (cols START..T)
                    nc.vector.tensor_copy(kT[:, START:T],
                                          kn_T[:, b * 16:(b + 1) * 16])

                    # V cast to bf16; patch 16 new rows (partition shift => DMA)
                    v_bf = vbp.tile([128, T], BF16, name=f"vbf_{b}", tag="vbf")
                    nc.vector.tensor_copy(v_bf[:], v_sb[:])
                    nc.sync.dma_start(
                        v_bf[112:128, (NT - 1) * 128:NT * 128],
                        xv_bf[col:col + 16, (b // 8) * 128:(b // 8 + 1) * 128])

                    # scores^T + exp  -> probs [t%128, (tt,q)]
                    probs = prp.tile([128, NT * 64], BF16, name=f"pr_{b}",
                                     tag="pr")
                    q_rhs = qv[:, :, b * 16:(b + 1) * 16]
                    for g2 in range(4):
                        sc = psB.tile([128, 512], F32, name=f"sc_{b}_{g2}",
                                      tag="b")
                        for j in range(8):
                            tt = g2 * 8 + j
                            nc.tensor.matmul(sc[:, j * 64:(j + 1) * 64],
                                             kT[:, tt * 128:(tt + 1) * 128],
                                             q_rhs, start=True, stop=True)
                        nc.scalar.activation(probs[:, g2 * 512:(g2 + 1) * 512],
                                             sc[:],
                                             mybir.ActivationFunctionType.Exp,
                                             scale=SCALE)

                    if DEBUG and b == 0:
                        dkT = stage.tile([128, T], F32, name="dkT", tag="st")
                        nc.vector.tensor_copy(dkT[:], kT[:])
                        nc.sync.dma_start(dbg_kT[:, :], dkT[:])
                        dpr = stage.tile([128, NT * 64], F32, name="dpr",
                                         tag="st")
                        nc.vector.tensor_copy(dpr[:, 0:NT * 64], probs[:])
                        nc.sync.dma_start(dbg_pr[:, :], dpr[:, 0:NT * 64])

                    # denominator: ones^T @ probs, then fold 8 column groups
                    dn_ps = psC.tile([128, 512], F32, name=f"dn_{b}", tag="c")
                    for j2 in range(4):
                        nc.tensor.matmul(dn_ps[:], ones16[:],
                                         probs[:, j2 * 512:(j2 + 1) * 512],
                                         start=(j2 == 0), stop=(j2 == 3))
                    d256 = dnp.tile([128, 256], F32, name=f"d256_{b}",
                                    tag="d256")
                    d128 = dnp.tile([128, 128], F32, name=f"d128_{b}",
                                    tag="d128")
                    d64 = dnp.tile([128, 64], F32, name=f"d64_{b}", tag="d64")
                    rcp = dnp.tile([128, 64], F32, name=f"rcp_{b}", tag="rcp")
                    # tensor_tensor cannot take two PSUM operands
                    dcp = dnp.tile([128, 256], F32, name=f"dcp_{b}", tag="dcp")
                    nc.vector.tensor_copy(dcp[:], dn_ps[:, 0:256])
                    nc.vector.tensor_add(d256[:], dcp[:], dn_ps[:, 256:512])
                    nc.vector.tensor_add(d128[:], d256[:, 0:128],
                                         d256[:, 128:256])
                    nc.vector.tensor_add(d64[:], d128[:, 0:64],
                                         d128[:, 64:128])
                    nc.vector.reciprocal(rcp[:], d64[:])

                    # attn_out^T = V^T @ probs  (accumulate over t-tiles)
                    at_ps = psC.tile([128, 64], F32, name=f"at_{b}", tag="c")
                    for tt in range(NT):
                        nc.tensor.matmul(at_ps[:],
                                         v_bf[:, tt * 128:(tt + 1) * 128],
                                         probs[:, tt * 64:(tt + 1) * 64],
                                         start=(tt == 0), stop=(tt == NT - 1))

                    # normalize + scatter into attnT[ch][h][:, col:col+16]
                    for h in range(HPC):
                        nc.vector.tensor_mul(
                            attnT[ch * HPC + h][:, col:col + 16],
                            at_ps[:, h * 16:(h + 1) * 16],
                            rcp[:, h * 16:(h + 1) * 16])

                def load_wo():
                    for hb in range(HPC):
                        wo_f = stage.tile([128, D], F32, name=f"wof_{hb}",
                                          tag="st")
                        nc.sync.dma_start(wo_f[:],
                                          woe[hb * 128:(hb + 1) * 128, :])
                        nc.vector.tensor_copy(wo_bf[:, hb * D:(hb + 1) * D],
                                              wo_f[:])

                def out_proj(ch):
                    for n in range(8):
                        y_ps = psC.tile([128, 512], F32, name=f"y_{ch}_{n}",
                                        tag="c")
                        for h in range(HPC):
                            nc.tensor.matmul(y_ps[:], attnT[ch * HPC + h][:],
                                             wo_bf[:, h * D + n * 512:
                                                   h * D + (n + 1) * 512],
                                             start=(h == 0), stop=(h == HPC - 1))
                        y_sb = ysb.tile([128, 512], F32, name=f"ysb_{ch}_{n}",
                                        tag="y")
                        nc.vector.tensor_copy(y_sb[:], y_ps[:])
                        nc.sync.dma_start(yb[ch][:, n * 512:(n + 1) * 512],
                                          y_sb[:])
                    nc.gpsimd.collective_compute(
                        "ReduceScatter",
                        mybir.AluOpType.add,
                        replica_groups=[list(range(N_CORES))],
                        ins=[yb[ch].opt()],
                        outs=[rs[ch].opt()],
                    )
                    nc.sync.dma_start(oute[ch * S:(ch + 1) * S, :],
                                      rs[ch][:, :])

                for b in range(8):
                    do_batch(b)
                load_wo()
                out_proj(0)
                for b in range(8, 16):
                    do_batch(b)
                out_proj(1)

                if DEBUG:
                    dat = stage.tile([128, 8 * 128], F32, name="dat", tag="st")
                    for i in range(8):
                        nc.vector.tensor_copy(dat[:, i * 128:(i + 1) * 128],
                                              attnT[i][:])
                    nc.sync.dma_start(dbg_at[:, :], dat[:, 0:8 * 128])

    nc.compile()
    return nc


def get_nc():
    if "nc" not in _CACHE:
        _CACHE["nc"] = _build()
    return _CACHE["nc"]


def make_in_maps(x, k, v, wq, wk, wv, wo, cache_k, cache_v,
                 freqs_cos, freqs_sin):
    f = np.float32
    x = np.ascontiguousarray(np.asarray(x, f).reshape(TOK, D))
    k = np.ascontiguousarray(np.asarray(k, f).reshape(TOK, D))
    v = np.ascontiguousarray(np.asarray(v, f).reshape(TOK, D))
    wq = np.asarray(wq, f)
    wk = np.asarray(wk, f)
    wv = np.asarray(wv, f)
    wo = np.asarray(wo, f)
    cache_k = np.asarray(cache_k, f)
    cache_v = np.asarray(cache_v, f)
    cos_t = np.ascontiguousarray(
        np.asarray(freqs_cos, f)[np.arange(128) % S])
    sin_t = np.ascontiguousarray(
        np.asarray(freqs_sin, f)[np.arange(128) % S])
    in_maps = []
    for c in range(N_CORES):
        in_maps.append({
            "x": x, "k": k, "v": v,
            "wq": np.ascontiguousarray(wq[:, c * QD:(c + 1) * QD]),
            "wk": np.ascontiguousarray(wk[:, c * HD:(c + 1) * HD]),
            "wv": np.ascontiguousarray(wv[:, c * HD:(c + 1) * HD]),
            "wo": np.ascontiguousarray(wo[c * QD:(c + 1) * QD, :]),
            "ck": np.ascontiguousarray(cache_k[:B, :T, c, :]),
            "cv": np.ascontiguousarray(cache_v[:B, :T, c, :]),
            "cos": cos_t, "sin": sin_t,
        })
    return in_maps


def assemble_output(results):
    out = np.empty((B, S, D), np.float32)
    for b in range(B):
        core = b % N_CORES
        r0 = 0 if b < 8 else S
        out[b] = results[core]["out"][r0:r0 + S, :]
    return out


def kernel(x, k, v, wq, wk, wv, wo, cache_k, cache_v,
           freqs_cos, freqs_sin, start_pos):
    assert int(start_pos) == START
    nc = get_nc()
    in_maps = make_in_maps(x, k, v, wq, wk, wv, wo, cache_k, cache_v,
                           freqs_cos, freqs_sin)
    res = run_bass_kernel_spmd(nc, in_maps, core_ids=list(range(N_CORES)))
    return assemble_output(res.results)
